# revision 25
# baseline (speedup 1.0000x reference)
"""DMoE layer kernel for Trainium2 (8 NeuronCores, data-parallel over batch).

Computation (per task t in 0..1):
    share_e = relu(x @ W_share[e])            e in 0..3   (shared experts)
    task_te = relu(x @ W_task[t,e])           e in 0..3   (task experts)
    gate_t  = softmax(x @ W_gate[t], axis=-1)             (8 weights)
    towers[t] = sum_e gate[t,:,e] * concat([share, task_t])[:, e, :]

Layout strategy (per core, 4096 rows):
  - Host pre-transposes x -> xT [256, 4096] so no on-chip transpose is needed.
  - All weights packed host-side into W_all [2(k-chunk), 128, 1552]:
    cols 0:512 shared experts, 512:1024 task0, 1024:1536 task1, 1536:1552 gates.
  - Per 128-row block: stationary = xT chunk (float32r), moving = W_all
    (float32r) -> PSUM [128, 1552]; full fp32-class precision at 1 cyc/row.
  - ACT: one wide exp over both tasks' gate logits, one wide ReLU pass
    PSUM->SBUF (fp16), and the gate normalization (copy-with-scale).
  - DVE: per-task softmax denominators (tensor_reduce) + reciprocal; 13 of
    the 16 (task, expert) combine terms as fused mul-add chains
    (scalar_tensor_tensor: out = R_e * gn_te + acc, gate as per-partition
    scalar).
  - GpSimd: the remaining 3 combine terms as tensor_tensor mult with the
    gate column broadcast along the free dim, plus both merge adds into
    the f32 towers. (GpSimd cannot execute TensorScalarPtr on TRN2.)
"""

import numpy as np

B, D_IN, H = 32768, 256, 128
N_TASK, N_EXP, N_SHARE = 2, 4, 4
N_CORES = 8
B_SHARD = B // N_CORES          # 4096
N_BLOCKS = B_SHARD // 128       # 32
NG = N_SHARE + N_EXP            # 8 gate cols per task
WCOLS = 512 * 3 + 2 * NG        # 1552

_CACHE = {}


def _build_program(acc_dt_name: str = "float16"):
    import concourse.bass as bass
    import concourse.mybir as mybir
    import concourse.tile as tile
    from concourse import bacc

    f32 = mybir.dt.float32
    f32r = mybir.dt.float32r
    acc_dt = getattr(mybir.dt, acc_dt_name)
    AF = mybir.ActivationFunctionType
    OP = mybir.AluOpType

    nc = bacc.Bacc("TRN2", target_bir_lowering=False)
    xT = nc.dram_tensor("xT", [D_IN, B_SHARD], f32r, kind="ExternalInput")
    wall = nc.dram_tensor("wall", [2, 128, WCOLS], f32r, kind="ExternalInput")
    outs = [
        nc.dram_tensor(f"out{i}", [N_TASK, 128, H], f32, kind="ExternalOutput")
        for i in range(N_BLOCKS)
    ]

    # xT rows d -> (k chunk, p partition)
    xT_v = xT.rearrange("(k p) b -> p k b", k=2)
    wall_v = wall.rearrange("k p c -> p k c")

    with tile.TileContext(nc) as tc:
        with (
            tc.tile_pool(name="wsb", bufs=1) as wpool,
            tc.tile_pool(name="xsb", bufs=1) as xpool,
            tc.tile_pool(name="epsum", bufs=2, space="PSUM") as epool,
            tc.tile_pool(name="gpsum", bufs=2, space="PSUM") as gpool,
            tc.tile_pool(name="relu", bufs=4) as rpool,
            tc.tile_pool(name="small", bufs=8) as spool,
            tc.tile_pool(name="accs", bufs=6) as apool,
            tc.tile_pool(name="outs", bufs=6) as opool,
        ):
            w_sb = wpool.tile([128, 2, WCOLS], f32r)
            nc.sync.dma_start(out=w_sb, in_=wall_v)

            # front-load all x tiles (unique buffers, no deps): keeps the
            # SP DMA sequencer from head-of-line blocking later x loads
            # behind output DMAs that wait on compute.
            x_tiles = []
            for i in range(N_BLOCKS):
                x_sb = xpool.tile([128, 2, 128], f32r, name=f"x{i}", tag=f"x{i}")
                nc.sync.dma_start(out=x_sb, in_=xT_v[:, :, bass.ts(i, 128)])
                x_tiles.append(x_sb)

            for i in range(N_BLOCKS):
                bs = bass.ts(i, 128)
                x_sb = x_tiles[i]

                ps_e = epool.tile([128, 1536], f32)
                ps_g = gpool.tile([128, 2 * NG], f32)

                for k in range(2):
                    lhsT = x_sb[:, k, :]
                    for j in range(3):
                        nc.tensor.matmul(
                            ps_e[:, bass.ts(j, 512)],
                            lhsT,
                            w_sb[:, k, bass.ts(j, 512)],
                            start=(k == 0),
                            stop=(k == 1),
                        )
                    nc.tensor.matmul(
                        ps_g,
                        lhsT,
                        w_sb[:, k, 1536:WCOLS],
                        start=(k == 0),
                        stop=(k == 1),
                    )

                # gates: one wide exp on ACT; per-task denominators on DVE
                expS = spool.tile([128, 2 * NG], f32)
                nc.scalar.activation(expS, ps_g, AF.Exp)
                den = spool.tile([128, 2], f32)
                nc.vector.tensor_reduce(
                    den,
                    expS.rearrange("p (t g) -> p t g", t=2),
                    axis=mybir.AxisListType.X,
                    op=OP.add,
                )
                rden = spool.tile([128, 2], f32)
                nc.vector.reciprocal(rden, den)
                # normalized gates: gn[:, t*8:(t+1)*8] = expS_t * rden_t
                # (on ACT: copy with per-partition scale; ACT has slack)
                gn = spool.tile([128, 2 * NG], f32)
                for t in range(2):
                    nc.scalar.mul(
                        gn[:, bass.ts(t, NG)],
                        expS[:, bass.ts(t, NG)],
                        rden[:, t : t + 1],
                    )

                # one wide relu pass PSUM->SBUF, fp16
                relu = rpool.tile([128, 1536], acc_dt)
                nc.scalar.activation(relu, ps_e, AF.Relu)

                # combine: towers[t] = sum_e gn_te * R_te
                # DVE: fused mul-add STT chains (1x, 194ns/term) for 12 terms.
                # GpSimd (no TensorScalarPtr support on HW) takes 4 terms as
                # tensor_tensor mult(+add) with the gate column broadcast
                # along the free dim, plus both merge adds into the f32 tower.
                tower = opool.tile([128, 2, H], f32)
                for t in range(2):
                    # expert column blocks for this task, in gate order:
                    # 4 shared (cols 0:512) then 4 task-specific
                    slices = [bass.ts(e, 128) for e in range(4)] + [
                        bass.ts(4 + 4 * t + e, 128) for e in range(4)
                    ]

                    n_dve = 7 if t == 0 else 6
                    # DVE chain over terms [0, n_dve)
                    a = [
                        apool.tile(
                            [128, 128], acc_dt, name=f"acc{t}{j}", tag=f"acc{t}{j}"
                        )
                        for j in range(2)
                    ]
                    nc.vector.tensor_scalar_mul(
                        a[0], relu[:, slices[0]], gn[:, t * NG : t * NG + 1]
                    )
                    c = 0
                    for e in range(1, n_dve):
                        nx = 1 - c
                        nc.vector.scalar_tensor_tensor(
                            out=a[nx],
                            in0=relu[:, slices[e]],
                            scalar=gn[:, t * NG + e : t * NG + e + 1],
                            in1=a[c],
                            op0=OP.mult,
                            op1=OP.add,
                        )
                        c = nx
                    h_dve = a[c]

                    # Pool: products for terms [n_dve, 8) via broadcast mult
                    ps = []
                    for e in range(n_dve, NG):
                        p = apool.tile(
                            [128, 128], acc_dt, name=f"pp{t}{e}", tag=f"pp{t}{e}"
                        )
                        r_in, g_in = bass.broadcast_tensor_aps(
                            relu[:, slices[e]],
                            gn[:, t * NG + e : t * NG + e + 1],
                        )
                        nc.gpsimd.tensor_tensor(out=p, in0=r_in, in1=g_in, op=OP.mult)
                        ps.append(p)
                    while len(ps) > 1:
                        q = apool.tile(
                            [128, 128],
                            acc_dt,
                            name=f"pq{t}{len(ps)}",
                            tag=f"pq{t}{len(ps)}",
                        )
                        nc.gpsimd.tensor_add(q, ps[0], ps[1])
                        ps = [q] + ps[2:]
                    # merge on Pool into the f32 tower
                    nc.gpsimd.tensor_add(tower[:, t, :], h_dve, ps[0])
                nc.sync.dma_start(
                    out=outs[i].rearrange("t b h -> b t h"), in_=tower
                )

    nc.compile()
    return nc


def _numpy_fallback(x, W_share, b_share, W_task, b_task, W_gate, b_gate):
    share = np.maximum(np.einsum("bd,edh->beh", x, W_share) + b_share, 0.0)
    task = np.maximum(
        np.einsum("bd,tedh->tbeh", x, W_task) + b_task[:, None], 0.0
    )
    logit = np.einsum("bd,tdg->tbg", x, W_gate) + b_gate[:, None]
    logit -= logit.max(axis=-1, keepdims=True)
    e = np.exp(logit)
    gate = e / e.sum(axis=-1, keepdims=True)
    share_b = np.broadcast_to(share[None], (N_TASK, x.shape[0], N_SHARE, H))
    experts = np.concatenate([share_b, task], axis=2)
    return np.einsum("tbeh,tbe->tbh", experts, gate).astype(np.float32)


def kernel(x, W_share, b_share, W_task, b_task, W_gate, b_gate):
    x = np.asarray(x, dtype=np.float32)
    W_share = np.asarray(W_share, dtype=np.float32)
    W_task = np.asarray(W_task, dtype=np.float32)
    W_gate = np.asarray(W_gate, dtype=np.float32)
    b_share = np.asarray(b_share, dtype=np.float32)
    b_task = np.asarray(b_task, dtype=np.float32)
    b_gate = np.asarray(b_gate, dtype=np.float32)

    if b_share.any() or b_task.any() or b_gate.any():
        # spec fills all biases with zeros; exact-but-slow fallback otherwise
        return _numpy_fallback(x, W_share, b_share, W_task, b_task, W_gate, b_gate)

    from concourse.bass_utils import run_bass_kernel_spmd

    if "nc" not in _CACHE:
        _CACHE["nc"] = _build_program()
    nc = _CACHE["nc"]

    # pack weights: [2 (k chunk), 128, 1552]
    wall = np.empty((2, 128, WCOLS), dtype=np.float32)
    for k in range(2):
        dk = slice(k * 128, (k + 1) * 128)
        wall[k, :, 0:512] = W_share.transpose(1, 0, 2).reshape(D_IN, 512)[dk]
        wall[k, :, 512:1024] = W_task[0].transpose(1, 0, 2).reshape(D_IN, 512)[dk]
        wall[k, :, 1024:1536] = W_task[1].transpose(1, 0, 2).reshape(D_IN, 512)[dk]
        wall[k, :, 1536 : 1536 + NG] = W_gate[0][dk]
        wall[k, :, 1536 + NG : WCOLS] = W_gate[1][dk]

    xT = np.ascontiguousarray(x.T)  # [256, 32768]

    in_maps = []
    for c in range(N_CORES):
        in_maps.append(
            {
                "xT": np.ascontiguousarray(xT[:, c * B_SHARD : (c + 1) * B_SHARD]),
                "wall": wall,
            }
        )

    res = run_bass_kernel_spmd(nc, in_maps, core_ids=list(range(N_CORES)))
    # per core: N_BLOCKS tensors out{i} of [2, 128, H] -> [2, 4096, H]
    per_core = [
        np.concatenate([r[f"out{i}"] for i in range(N_BLOCKS)], axis=1)
        for r in res.results
    ]
    return np.concatenate(per_core, axis=1)


# revision 29
# speedup vs baseline: 1.0250x; 1.0250x over previous
"""DMoE layer kernel for Trainium2 (8 NeuronCores, data-parallel over batch).

Computation (per task t in 0..1):
    share_e = relu(x @ W_share[e])            e in 0..3   (shared experts)
    task_te = relu(x @ W_task[t,e])           e in 0..3   (task experts)
    gate_t  = softmax(x @ W_gate[t], axis=-1)             (8 weights)
    towers[t] = sum_e gate[t,:,e] * concat([share, task_t])[:, e, :]

Layout strategy (per core, 4096 rows):
  - Host pre-transposes x -> xT [256, 4096] so no on-chip transpose is needed.
  - All weights packed host-side into W_all [2(k-chunk), 128, 1552]:
    cols 0:512 shared experts, 512:1024 task0, 1024:1536 task1, 1536:1552 gates.
  - Per 128-row block: stationary = xT chunk (float32r), moving = W_all
    (float32r) -> PSUM [128, 1552]; full fp32-class precision at 1 cyc/row.
  - ACT: one wide exp over both tasks' gate logits, one wide ReLU pass
    PSUM->SBUF (fp16), and the gate normalization (copy-with-scale).
  - DVE: per-task softmax denominators (tensor_reduce) + reciprocal; 13 of
    the 16 (task, expert) combine terms as fused mul-add chains
    (scalar_tensor_tensor: out = R_e * gn_te + acc, gate as per-partition
    scalar).
  - GpSimd: the remaining 3 combine terms as tensor_tensor mult with the
    gate column broadcast along the free dim, plus both merge adds into
    the f32 towers. (GpSimd cannot execute TensorScalarPtr on TRN2.)
"""

import numpy as np

B, D_IN, H = 32768, 256, 128
N_TASK, N_EXP, N_SHARE = 2, 4, 4
N_CORES = 8
B_SHARD = B // N_CORES          # 4096
N_BLOCKS = B_SHARD // 128       # 32
NG = N_SHARE + N_EXP            # 8 gate cols per task
WCOLS = 512 * 3 + 2 * NG        # 1552

_CACHE = {}


def _build_program(acc_dt_name: str = "float16"):
    import concourse.bass as bass
    import concourse.mybir as mybir
    import concourse.tile as tile
    from concourse import bacc

    f32 = mybir.dt.float32
    f32r = mybir.dt.float32r
    acc_dt = getattr(mybir.dt, acc_dt_name)
    AF = mybir.ActivationFunctionType
    OP = mybir.AluOpType

    nc = bacc.Bacc("TRN2", target_bir_lowering=False)
    xT = nc.dram_tensor("xT", [D_IN, B_SHARD], f32r, kind="ExternalInput")
    wall = nc.dram_tensor("wall", [2, 128, WCOLS], f32r, kind="ExternalInput")
    outs = [
        nc.dram_tensor(f"out{i}", [N_TASK, 128, H], f32, kind="ExternalOutput")
        for i in range(N_BLOCKS)
    ]

    # xT rows d -> (k chunk, p partition)
    xT_v = xT.rearrange("(k p) b -> p k b", k=2)
    wall_v = wall.rearrange("k p c -> p k c")

    with tile.TileContext(nc) as tc:
        with (
            tc.tile_pool(name="wsb", bufs=1) as wpool,
            tc.tile_pool(name="xsb", bufs=1) as xpool,
            tc.tile_pool(name="epsum", bufs=2, space="PSUM") as epool,
            tc.tile_pool(name="gpsum", bufs=2, space="PSUM") as gpool,
            tc.tile_pool(name="relu", bufs=6) as rpool,
            tc.tile_pool(name="small", bufs=12) as spool,
            tc.tile_pool(name="accs", bufs=8) as apool,
            tc.tile_pool(name="outs", bufs=8) as opool,
        ):
            # warm up the ACT exp table set during the initial weight DMA
            # (first table load costs ~2.7us and otherwise lands on the
            # first block's critical path)
            warm = spool.tile([1, 1], f32, name="warm", tag="warm")
            nc.vector.memset(warm, 0.0)
            nc.scalar.activation(warm, warm, AF.Exp)

            w_sb = wpool.tile([128, 2, WCOLS], f32r)
            # split the weight load by k-chunk: halves land on parallel DMA
            # queues and the k=0 matmuls only wait on the first half
            for k in range(2):
                nc.sync.dma_start(out=w_sb[:, k, :], in_=wall_v[:, k, :])

            # front-load all x tiles (unique buffers, no deps): keeps the
            # SP DMA sequencer from head-of-line blocking later x loads
            # behind output DMAs that wait on compute.
            x_tiles = []
            for i in range(N_BLOCKS):
                x_sb = xpool.tile([128, 2, 128], f32r, name=f"x{i}", tag=f"x{i}")
                nc.sync.dma_start(out=x_sb, in_=xT_v[:, :, bass.ts(i, 128)])
                x_tiles.append(x_sb)

            for i in range(N_BLOCKS):
                bs = bass.ts(i, 128)
                x_sb = x_tiles[i]

                ps_e = epool.tile([128, 1536], f32)
                ps_g = gpool.tile([128, 2 * NG], f32)

                for k in range(2):
                    lhsT = x_sb[:, k, :]
                    for j in range(3):
                        nc.tensor.matmul(
                            ps_e[:, bass.ts(j, 512)],
                            lhsT,
                            w_sb[:, k, bass.ts(j, 512)],
                            start=(k == 0),
                            stop=(k == 1),
                        )
                    nc.tensor.matmul(
                        ps_g,
                        lhsT,
                        w_sb[:, k, 1536:WCOLS],
                        start=(k == 0),
                        stop=(k == 1),
                    )

                # gates: one wide exp on ACT; per-task denominators on DVE
                expS = spool.tile([128, 2 * NG], f32)
                nc.scalar.activation(expS, ps_g, AF.Exp)
                den = spool.tile([128, 2], f32)
                nc.vector.tensor_reduce(
                    den,
                    expS.rearrange("p (t g) -> p t g", t=2),
                    axis=mybir.AxisListType.X,
                    op=OP.add,
                )
                rden = spool.tile([128, 2], f32)
                nc.vector.reciprocal(rden, den)
                # normalized gates: gn[:, t*8:(t+1)*8] = expS_t * rden_t
                # (on ACT: copy with per-partition scale; ACT has slack)
                gn = spool.tile([128, 2 * NG], f32)
                for t in range(2):
                    nc.scalar.mul(
                        gn[:, bass.ts(t, NG)],
                        expS[:, bass.ts(t, NG)],
                        rden[:, t : t + 1],
                    )

                # one wide relu pass PSUM->SBUF, fp16
                relu = rpool.tile([128, 1536], acc_dt)
                nc.scalar.activation(relu, ps_e, AF.Relu)

                # combine: towers[t] = sum_e gn_te * R_te
                # DVE: fused mul-add STT chains (1x, ~194ns/term) for 13 terms.
                # GpSimd (no TensorScalarPtr support on HW) takes 3 terms as
                # tensor_tensor mult(+add) with the gate column broadcast
                # along the free dim, plus both merge adds into the f32 tower.
                tower = opool.tile([128, 2, H], f32)
                for t in range(2):
                    # expert column blocks for this task, in gate order:
                    # 4 shared (cols 0:512) then 4 task-specific
                    slices = [bass.ts(e, 128) for e in range(4)] + [
                        bass.ts(4 + 4 * t + e, 128) for e in range(4)
                    ]

                    n_dve = 7 if t == 0 else 6
                    # DVE chain over terms [0, n_dve)
                    a = [
                        apool.tile(
                            [128, 128], acc_dt, name=f"acc{t}{j}", tag=f"acc{t}{j}"
                        )
                        for j in range(2)
                    ]
                    nc.vector.tensor_scalar_mul(
                        a[0], relu[:, slices[0]], gn[:, t * NG : t * NG + 1]
                    )
                    c = 0
                    for e in range(1, n_dve):
                        nx = 1 - c
                        nc.vector.scalar_tensor_tensor(
                            out=a[nx],
                            in0=relu[:, slices[e]],
                            scalar=gn[:, t * NG + e : t * NG + e + 1],
                            in1=a[c],
                            op0=OP.mult,
                            op1=OP.add,
                        )
                        c = nx
                    h_dve = a[c]

                    # Pool: products for terms [n_dve, 8) via broadcast mult
                    ps = []
                    for e in range(n_dve, NG):
                        p = apool.tile(
                            [128, 128], acc_dt, name=f"pp{t}{e}", tag=f"pp{t}{e}"
                        )
                        r_in, g_in = bass.broadcast_tensor_aps(
                            relu[:, slices[e]],
                            gn[:, t * NG + e : t * NG + e + 1],
                        )
                        nc.gpsimd.tensor_tensor(out=p, in0=r_in, in1=g_in, op=OP.mult)
                        ps.append(p)
                    while len(ps) > 1:
                        q = apool.tile(
                            [128, 128],
                            acc_dt,
                            name=f"pq{t}{len(ps)}",
                            tag=f"pq{t}{len(ps)}",
                        )
                        nc.gpsimd.tensor_add(q, ps[0], ps[1])
                        ps = [q] + ps[2:]
                    # merge on Pool into the f32 tower
                    nc.gpsimd.tensor_add(tower[:, t, :], h_dve, ps[0])
                nc.sync.dma_start(
                    out=outs[i].rearrange("t b h -> b t h"), in_=tower
                )

    nc.compile()
    return nc


def _numpy_fallback(x, W_share, b_share, W_task, b_task, W_gate, b_gate):
    share = np.maximum(np.einsum("bd,edh->beh", x, W_share) + b_share, 0.0)
    task = np.maximum(
        np.einsum("bd,tedh->tbeh", x, W_task) + b_task[:, None], 0.0
    )
    logit = np.einsum("bd,tdg->tbg", x, W_gate) + b_gate[:, None]
    logit -= logit.max(axis=-1, keepdims=True)
    e = np.exp(logit)
    gate = e / e.sum(axis=-1, keepdims=True)
    share_b = np.broadcast_to(share[None], (N_TASK, x.shape[0], N_SHARE, H))
    experts = np.concatenate([share_b, task], axis=2)
    return np.einsum("tbeh,tbe->tbh", experts, gate).astype(np.float32)


def kernel(x, W_share, b_share, W_task, b_task, W_gate, b_gate):
    x = np.asarray(x, dtype=np.float32)
    W_share = np.asarray(W_share, dtype=np.float32)
    W_task = np.asarray(W_task, dtype=np.float32)
    W_gate = np.asarray(W_gate, dtype=np.float32)
    b_share = np.asarray(b_share, dtype=np.float32)
    b_task = np.asarray(b_task, dtype=np.float32)
    b_gate = np.asarray(b_gate, dtype=np.float32)

    if b_share.any() or b_task.any() or b_gate.any():
        # spec fills all biases with zeros; exact-but-slow fallback otherwise
        return _numpy_fallback(x, W_share, b_share, W_task, b_task, W_gate, b_gate)

    from concourse.bass_utils import run_bass_kernel_spmd

    if "nc" not in _CACHE:
        _CACHE["nc"] = _build_program()
    nc = _CACHE["nc"]

    # pack weights: [2 (k chunk), 128, 1552]
    wall = np.empty((2, 128, WCOLS), dtype=np.float32)
    for k in range(2):
        dk = slice(k * 128, (k + 1) * 128)
        wall[k, :, 0:512] = W_share.transpose(1, 0, 2).reshape(D_IN, 512)[dk]
        wall[k, :, 512:1024] = W_task[0].transpose(1, 0, 2).reshape(D_IN, 512)[dk]
        wall[k, :, 1024:1536] = W_task[1].transpose(1, 0, 2).reshape(D_IN, 512)[dk]
        wall[k, :, 1536 : 1536 + NG] = W_gate[0][dk]
        wall[k, :, 1536 + NG : WCOLS] = W_gate[1][dk]

    xT = np.ascontiguousarray(x.T)  # [256, 32768]

    in_maps = []
    for c in range(N_CORES):
        in_maps.append(
            {
                "xT": np.ascontiguousarray(xT[:, c * B_SHARD : (c + 1) * B_SHARD]),
                "wall": wall,
            }
        )

    res = run_bass_kernel_spmd(nc, in_maps, core_ids=list(range(N_CORES)))
    # per core: N_BLOCKS tensors out{i} of [2, 128, H] -> [2, 4096, H]
    per_core = [
        np.concatenate([r[f"out{i}"] for i in range(N_BLOCKS)], axis=1)
        for r in res.results
    ]
    return np.concatenate(per_core, axis=1)


# revision 40
# speedup vs baseline: 1.0905x; 1.0639x over previous
"""DMoE layer kernel for Trainium2 (8 NeuronCores, data-parallel over batch).

Computation (per task t in 0..1):
    share_e = relu(x @ W_share[e])            e in 0..3   (shared experts)
    task_te = relu(x @ W_task[t,e])           e in 0..3   (task experts)
    gate_t  = softmax(x @ W_gate[t], axis=-1)             (8 weights)
    towers[t] = sum_e gate[t,:,e] * concat([share, task_t])[:, e, :]

Layout strategy (per core, 4096 rows):
  - Host pre-transposes x -> xT [256, 4096] so no on-chip transpose is needed.
  - All weights packed host-side into W_all [2(k-chunk), 128, 1552]:
    cols 0:512 shared experts, 512:1024 task0, 1024:1536 task1, 1536:1552 gates.
  - Per 128-row block: stationary = xT chunk (float32r), moving = W_all
    (float32r) -> PSUM [128, 1552]; full fp32-class precision at 1 cyc/row.
  - ACT: one wide exp over both tasks' gate logits, one wide ReLU pass
    PSUM->SBUF (fp16), and the gate normalization (copy-with-scale).
  - DVE: per-task softmax denominators (tensor_reduce) + reciprocal; 13 of
    the 16 (task, expert) combine terms as fused mul-add chains
    (scalar_tensor_tensor: out = R_e * gn_te + acc, gate as per-partition
    scalar).
  - GpSimd: the remaining 3 combine terms as tensor_tensor mult with the
    gate column broadcast along the free dim, plus both merge adds into
    the f32 towers. (GpSimd cannot execute TensorScalarPtr on TRN2.)
"""

import numpy as np

B, D_IN, H = 32768, 256, 128
N_TASK, N_EXP, N_SHARE = 2, 4, 4
N_CORES = 8
B_SHARD = B // N_CORES          # 4096
N_BLOCKS = B_SHARD // 128       # 32
NG = N_SHARE + N_EXP            # 8 gate cols per task
WCOLS = 512 * 3 + 2 * NG        # 1552

_CACHE = {}


def _build_program(acc_dt_name: str = "float16"):
    import concourse.bass as bass
    import concourse.mybir as mybir
    import concourse.tile as tile
    from concourse import bacc

    f32 = mybir.dt.float32
    f32r = mybir.dt.float32r
    acc_dt = getattr(mybir.dt, acc_dt_name)
    AF = mybir.ActivationFunctionType
    OP = mybir.AluOpType

    nc = bacc.Bacc("TRN2", target_bir_lowering=False)
    xT = nc.dram_tensor("xT", [D_IN, B_SHARD], f32r, kind="ExternalInput")
    wall = nc.dram_tensor("wall", [2, 128, WCOLS], f32r, kind="ExternalInput")
    outs = [
        nc.dram_tensor(f"out{i}", [N_TASK, 128, H], f32, kind="ExternalOutput")
        for i in range(N_BLOCKS)
    ]

    # xT rows d -> (k chunk, p partition)
    xT_v = xT.rearrange("(k p) b -> p k b", k=2)
    wall_v = wall.rearrange("k p c -> p k c")

    with tile.TileContext(nc) as tc:
        with (
            tc.tile_pool(name="wsb", bufs=1) as wpool,
            tc.tile_pool(name="xsb", bufs=1) as xpool,
            tc.tile_pool(name="epsum", bufs=2, space="PSUM") as epool,
            tc.tile_pool(name="gpsum", bufs=2, space="PSUM") as gpool,
            tc.tile_pool(name="relu", bufs=6) as rpool,
            tc.tile_pool(name="small", bufs=12) as spool,
            tc.tile_pool(name="accs", bufs=8) as apool,
            tc.tile_pool(name="outs", bufs=8) as opool,
        ):
            w_sb = wpool.tile([128, 2, WCOLS], f32r)
            # split the weight load into per-k, per-column-group DMAs that
            # match the matmul consumers: the first matmul only waits on its
            # own 256KB chunk instead of the whole 1.6MB load
            # ACT exp-table warmup: the ~2.7us table load overlaps the
            # weight DMAs instead of landing on block 0's critical path
            warm = spool.tile([1, 1], f32, name="warm", tag="warm")
            nc.vector.memset(warm, 0.0)
            nc.scalar.activation(warm, warm, AF.Exp)

            # PE clock warmup: short matmuls on a const tile while the
            # weight DMAs stream, so block 0's real matmuls run warm
            pwarm = spool.tile([1, 128], f32, name="pwarm", tag="pwarm")
            nc.vector.memset(pwarm, 1.0)
            ps_w = epool.tile([1, 128], f32, name="ps_e", tag="ps_e")
            for _ in range(10):
                nc.tensor.matmul(
                    ps_w, pwarm[0:1, 0:1], pwarm, start=True, stop=True
                )

            # weight chunks split across the ACT HWDGE ring and the GpSimd
            # SWDGE (both idle at start) so they stream in parallel with the
            # x tiles on the SP ring; chunk order matches consumer order
            for idx, (k, (c0, c1)) in enumerate(
                (k, c)
                for k in range(2)
                for c in ((0, 512), (512, 1024), (1024, WCOLS))
            ):
                eng = nc.scalar if idx % 2 == 0 else nc.gpsimd
                eng.dma_start(out=w_sb[:, k, c0:c1], in_=wall_v[:, k, c0:c1])


            # front-load all x tiles (unique buffers, no deps): keeps the
            # SP DMA sequencer from head-of-line blocking later x loads
            # behind output DMAs that wait on compute.
            x_tiles = []
            for i in range(N_BLOCKS):
                x_sb = xpool.tile([128, 2, 128], f32r, name=f"x{i}", tag=f"x{i}")
                nc.sync.dma_start(out=x_sb, in_=xT_v[:, :, bass.ts(i, 128)])
                x_tiles.append(x_sb)

            for i in range(N_BLOCKS):
                bs = bass.ts(i, 128)
                x_sb = x_tiles[i]

                ps_e = epool.tile([128, 1536], f32)
                ps_g = gpool.tile([128, 2 * NG], f32)

                for k in range(2):
                    lhsT = x_sb[:, k, :]
                    for j in range(3):
                        nc.tensor.matmul(
                            ps_e[:, bass.ts(j, 512)],
                            lhsT,
                            w_sb[:, k, bass.ts(j, 512)],
                            start=(k == 0),
                            stop=(k == 1),
                        )
                    nc.tensor.matmul(
                        ps_g,
                        lhsT,
                        w_sb[:, k, 1536:WCOLS],
                        start=(k == 0),
                        stop=(k == 1),
                    )

                # gates: one wide exp on ACT; per-task denominators on DVE
                expS = spool.tile([128, 2 * NG], f32)
                nc.scalar.activation(expS, ps_g, AF.Exp)
                den = spool.tile([128, 2], f32)
                nc.vector.tensor_reduce(
                    den,
                    expS.rearrange("p (t g) -> p t g", t=2),
                    axis=mybir.AxisListType.X,
                    op=OP.add,
                )
                rden = spool.tile([128, 2], f32)
                nc.vector.reciprocal(rden, den)
                # normalized gates: gn[:, t*8:(t+1)*8] = expS_t * rden_t
                # (on ACT: copy with per-partition scale; ACT has slack)
                gn = spool.tile([128, 2 * NG], f32)
                for t in range(2):
                    nc.scalar.mul(
                        gn[:, bass.ts(t, NG)],
                        expS[:, bass.ts(t, NG)],
                        rden[:, t : t + 1],
                    )

                # one wide relu pass PSUM->SBUF, fp16
                relu = rpool.tile([128, 1536], acc_dt)
                nc.scalar.activation(relu, ps_e, AF.Relu)

                # combine: towers[t] = sum_e gn_te * R_te
                # DVE: fused mul-add STT chains (1x, ~194ns/term) for 13 terms.
                # GpSimd (no TensorScalarPtr support on HW) takes 3 terms as
                # tensor_tensor mult(+add) with the gate column broadcast
                # along the free dim, plus both merge adds into the f32 tower.
                tower = opool.tile([128, 2, H], f32)
                for t in range(2):
                    # expert column blocks for this task, in gate order:
                    # 4 shared (cols 0:512) then 4 task-specific
                    slices = [bass.ts(e, 128) for e in range(4)] + [
                        bass.ts(4 + 4 * t + e, 128) for e in range(4)
                    ]

                    n_dve = 7 if t == 0 else 6
                    # DVE chain over terms [0, n_dve)
                    a = [
                        apool.tile(
                            [128, 128], acc_dt, name=f"acc{t}{j}", tag=f"acc{t}{j}"
                        )
                        for j in range(2)
                    ]
                    nc.vector.tensor_scalar_mul(
                        a[0], relu[:, slices[0]], gn[:, t * NG : t * NG + 1]
                    )
                    c = 0
                    for e in range(1, n_dve):
                        nx = 1 - c
                        nc.vector.scalar_tensor_tensor(
                            out=a[nx],
                            in0=relu[:, slices[e]],
                            scalar=gn[:, t * NG + e : t * NG + e + 1],
                            in1=a[c],
                            op0=OP.mult,
                            op1=OP.add,
                        )
                        c = nx
                    h_dve = a[c]

                    # Pool: products for terms [n_dve, 8) via broadcast mult
                    ps = []
                    for e in range(n_dve, NG):
                        p = apool.tile(
                            [128, 128], acc_dt, name=f"pp{t}{e}", tag=f"pp{t}{e}"
                        )
                        r_in, g_in = bass.broadcast_tensor_aps(
                            relu[:, slices[e]],
                            gn[:, t * NG + e : t * NG + e + 1],
                        )
                        nc.gpsimd.tensor_tensor(out=p, in0=r_in, in1=g_in, op=OP.mult)
                        ps.append(p)
                    while len(ps) > 1:
                        q = apool.tile(
                            [128, 128],
                            acc_dt,
                            name=f"pq{t}{len(ps)}",
                            tag=f"pq{t}{len(ps)}",
                        )
                        nc.gpsimd.tensor_add(q, ps[0], ps[1])
                        ps = [q] + ps[2:]
                    # merge on Pool into the f32 tower
                    nc.gpsimd.tensor_add(tower[:, t, :], h_dve, ps[0])
                nc.sync.dma_start(
                    out=outs[i].rearrange("t b h -> b t h"), in_=tower
                )

    nc.compile()
    return nc


def _numpy_fallback(x, W_share, b_share, W_task, b_task, W_gate, b_gate):
    share = np.maximum(np.einsum("bd,edh->beh", x, W_share) + b_share, 0.0)
    task = np.maximum(
        np.einsum("bd,tedh->tbeh", x, W_task) + b_task[:, None], 0.0
    )
    logit = np.einsum("bd,tdg->tbg", x, W_gate) + b_gate[:, None]
    logit -= logit.max(axis=-1, keepdims=True)
    e = np.exp(logit)
    gate = e / e.sum(axis=-1, keepdims=True)
    share_b = np.broadcast_to(share[None], (N_TASK, x.shape[0], N_SHARE, H))
    experts = np.concatenate([share_b, task], axis=2)
    return np.einsum("tbeh,tbe->tbh", experts, gate).astype(np.float32)


def kernel(x, W_share, b_share, W_task, b_task, W_gate, b_gate):
    x = np.asarray(x, dtype=np.float32)
    W_share = np.asarray(W_share, dtype=np.float32)
    W_task = np.asarray(W_task, dtype=np.float32)
    W_gate = np.asarray(W_gate, dtype=np.float32)
    b_share = np.asarray(b_share, dtype=np.float32)
    b_task = np.asarray(b_task, dtype=np.float32)
    b_gate = np.asarray(b_gate, dtype=np.float32)

    if b_share.any() or b_task.any() or b_gate.any():
        # spec fills all biases with zeros; exact-but-slow fallback otherwise
        return _numpy_fallback(x, W_share, b_share, W_task, b_task, W_gate, b_gate)

    from concourse.bass_utils import run_bass_kernel_spmd

    if "nc" not in _CACHE:
        _CACHE["nc"] = _build_program()
    nc = _CACHE["nc"]

    # pack weights: [2 (k chunk), 128, 1552]
    wall = np.empty((2, 128, WCOLS), dtype=np.float32)
    for k in range(2):
        dk = slice(k * 128, (k + 1) * 128)
        wall[k, :, 0:512] = W_share.transpose(1, 0, 2).reshape(D_IN, 512)[dk]
        wall[k, :, 512:1024] = W_task[0].transpose(1, 0, 2).reshape(D_IN, 512)[dk]
        wall[k, :, 1024:1536] = W_task[1].transpose(1, 0, 2).reshape(D_IN, 512)[dk]
        wall[k, :, 1536 : 1536 + NG] = W_gate[0][dk]
        wall[k, :, 1536 + NG : WCOLS] = W_gate[1][dk]

    xT = np.ascontiguousarray(x.T)  # [256, 32768]

    in_maps = []
    for c in range(N_CORES):
        in_maps.append(
            {
                "xT": np.ascontiguousarray(xT[:, c * B_SHARD : (c + 1) * B_SHARD]),
                "wall": wall,
            }
        )

    res = run_bass_kernel_spmd(nc, in_maps, core_ids=list(range(N_CORES)))
    # per core: N_BLOCKS tensors out{i} of [2, 128, H] -> [2, 4096, H]
    per_core = [
        np.concatenate([r[f"out{i}"] for i in range(N_BLOCKS)], axis=1)
        for r in res.results
    ]
    return np.concatenate(per_core, axis=1)


# revision 49
# speedup vs baseline: 1.1057x; 1.0139x over previous
"""DMoE layer kernel for Trainium2 (8 NeuronCores, data-parallel over batch).

Computation (per task t in 0..1):
    share_e = relu(x @ W_share[e])            e in 0..3   (shared experts)
    task_te = relu(x @ W_task[t,e])           e in 0..3   (task experts)
    gate_t  = softmax(x @ W_gate[t], axis=-1)             (8 weights)
    towers[t] = sum_e gate[t,:,e] * concat([share, task_t])[:, e, :]

Layout strategy (per core, 4096 rows):
  - Host pre-transposes x -> xT [256, 4096] so no on-chip transpose is needed.
  - All weights packed host-side into W_all [2(k-chunk), 128, 1552]:
    cols 0:512 shared experts, 512:1024 task0, 1024:1536 task1, 1536:1552 gates.
  - Per 128-row block: stationary = xT chunk (float32r), moving = W_all
    (float32r) -> PSUM [128, 1552]; full fp32-class precision at 1 cyc/row.
  - ACT: one wide exp over both tasks' gate logits, one wide ReLU pass
    PSUM->SBUF (fp16), and the gate normalization (copy-with-scale).
  - DVE: per-task softmax denominators (tensor_reduce) + reciprocal; 13 of
    the 16 (task, expert) combine terms as fused mul-add chains
    (scalar_tensor_tensor: out = R_e * gn_te + acc, gate as per-partition
    scalar).
  - GpSimd: the remaining 3 combine terms as tensor_tensor mult with the
    gate column broadcast along the free dim, plus both merge adds into
    the f32 towers. (GpSimd cannot execute TensorScalarPtr on TRN2.)
"""

import numpy as np

B, D_IN, H = 32768, 256, 128
N_TASK, N_EXP, N_SHARE = 2, 4, 4
N_CORES = 8
B_SHARD = B // N_CORES          # 4096
N_BLOCKS = B_SHARD // 128       # 32
NG = N_SHARE + N_EXP            # 8 gate cols per task
WCOLS = 512 * 3 + 2 * NG        # 1552

_CACHE = {}


def _build_program(acc_dt_name: str = "float16"):
    import concourse.bass as bass
    import concourse.mybir as mybir
    import concourse.tile as tile
    from concourse import bacc

    f32 = mybir.dt.float32
    f32r = mybir.dt.float32r
    acc_dt = getattr(mybir.dt, acc_dt_name)
    AF = mybir.ActivationFunctionType
    OP = mybir.AluOpType

    nc = bacc.Bacc("TRN2", target_bir_lowering=False)
    xT = nc.dram_tensor("xT", [D_IN, B_SHARD], f32r, kind="ExternalInput")
    wall = nc.dram_tensor("wall", [2, 128, WCOLS], f32r, kind="ExternalInput")
    outs = [
        nc.dram_tensor(f"out{i}", [N_TASK, 128, H], f32, kind="ExternalOutput")
        for i in range(N_BLOCKS)
    ]

    # xT rows d -> (k chunk, p partition)
    xT_v = xT.rearrange("(k p) b -> p k b", k=2)
    wall_v = wall.rearrange("k p c -> p k c")

    with tile.TileContext(nc) as tc:
        with (
            tc.tile_pool(name="wsb", bufs=1) as wpool,
            tc.tile_pool(name="xsb", bufs=1) as xpool,
            tc.tile_pool(name="epsum", bufs=2, space="PSUM") as epool,
            tc.tile_pool(name="gpsum", bufs=2, space="PSUM") as gpool,
            tc.tile_pool(name="relu", bufs=8) as rpool,
            tc.tile_pool(name="small", bufs=16) as spool,
            tc.tile_pool(name="accs", bufs=10) as apool,
            tc.tile_pool(name="outs", bufs=10) as opool,
        ):
            w_sb = wpool.tile([128, 2, WCOLS], f32r)
            # split the weight load into per-k, per-column-group DMAs that
            # match the matmul consumers: the first matmul only waits on its
            # own 256KB chunk instead of the whole 1.6MB load
            # ACT exp-table warmup: the ~2.7us table load overlaps the
            # weight DMAs instead of landing on block 0's critical path
            warm = spool.tile([1, 1], f32, name="warm", tag="warm")
            nc.vector.memset(warm, 0.0)
            nc.scalar.activation(warm, warm, AF.Exp)

            # PE clock warmup: short matmuls on a const tile while the
            # weight DMAs stream, so block 0's real matmuls run warm
            pwarm = spool.tile([1, 128], f32, name="pwarm", tag="pwarm")
            nc.vector.memset(pwarm, 1.0)
            ps_w = epool.tile([1, 128], f32, name="ps_e", tag="ps_e")
            for _ in range(10):
                nc.tensor.matmul(
                    ps_w, pwarm[0:1, 0:1], pwarm, start=True, stop=True
                )

            # weight chunks split across the ACT HWDGE ring and the GpSimd
            # SWDGE (both idle at start) so they stream in parallel with the
            # x tiles on the SP ring; chunk order matches consumer order
            for idx, (k, (c0, c1)) in enumerate(
                (k, c)
                for k in range(2)
                for c in ((0, 512), (512, 1024), (1024, WCOLS))
            ):
                eng = nc.scalar if idx % 2 == 0 else nc.gpsimd
                eng.dma_start(out=w_sb[:, k, c0:c1], in_=wall_v[:, k, c0:c1])


            # front-load all x tiles (unique buffers, no deps): keeps the
            # SP DMA sequencer from head-of-line blocking later x loads
            # behind output DMAs that wait on compute.
            x_tiles = []
            for i in range(N_BLOCKS):
                x_sb = xpool.tile([128, 2, 128], f32r, name=f"x{i}", tag=f"x{i}")
                nc.sync.dma_start(out=x_sb, in_=xT_v[:, :, bass.ts(i, 128)])
                x_tiles.append(x_sb)

            for i in range(N_BLOCKS):
                bs = bass.ts(i, 128)
                x_sb = x_tiles[i]

                ps_e = epool.tile([128, 1536], f32)
                ps_g = gpool.tile([128, 2 * NG], f32)

                for k in range(2):
                    lhsT = x_sb[:, k, :]
                    nc.tensor.matmul(
                        ps_g,
                        lhsT,
                        w_sb[:, k, 1536:WCOLS],
                        start=(k == 0),
                        stop=(k == 1),
                    )
                    for j in range(3):
                        nc.tensor.matmul(
                            ps_e[:, bass.ts(j, 512)],
                            lhsT,
                            w_sb[:, k, bass.ts(j, 512)],
                            start=(k == 0),
                            stop=(k == 1),
                        )

                # gates: one wide exp on ACT; per-task denominators on DVE
                expS = spool.tile([128, 2 * NG], f32)
                nc.scalar.activation(expS, ps_g, AF.Exp)
                den = spool.tile([128, 2], f32)
                nc.vector.tensor_reduce(
                    den,
                    expS.rearrange("p (t g) -> p t g", t=2),
                    axis=mybir.AxisListType.X,
                    op=OP.add,
                )
                rden = spool.tile([128, 2], f32)
                nc.vector.reciprocal(rden, den)
                # normalized gates: gn[:, t*8:(t+1)*8] = expS_t * rden_t
                # (on ACT: copy with per-partition scale; ACT has slack)
                gn = spool.tile([128, 2 * NG], f32)
                for t in range(2):
                    nc.scalar.mul(
                        gn[:, bass.ts(t, NG)],
                        expS[:, bass.ts(t, NG)],
                        rden[:, t : t + 1],
                    )

                # one wide relu pass PSUM->SBUF, fp16
                relu = rpool.tile([128, 1536], acc_dt)
                nc.scalar.activation(relu, ps_e, AF.Relu)

                # combine: towers[t] = sum_e gn_te * R_te
                # DVE: fused mul-add STT chains (1x, ~194ns/term) for 13 terms.
                # GpSimd (no TensorScalarPtr support on HW) takes 3 terms as
                # tensor_tensor mult(+add) with the gate column broadcast
                # along the free dim, plus both merge adds into the f32 tower.
                tower = opool.tile([128, 2, H], f32, name="tower", tag="tower")
                for t in range(2):
                    # expert column blocks for this task, in gate order:
                    # 4 shared (cols 0:512) then 4 task-specific
                    slices = [bass.ts(e, 128) for e in range(4)] + [
                        bass.ts(4 + 4 * t + e, 128) for e in range(4)
                    ]

                    n_dve = 7 if t == 0 else 6
                    # DVE chain over terms [0, n_dve)
                    a = [
                        apool.tile(
                            [128, 128], acc_dt, name=f"acc{t}{j}", tag=f"acc{t}{j}"
                        )
                        for j in range(2)
                    ]
                    nc.vector.tensor_scalar_mul(
                        a[0], relu[:, slices[0]], gn[:, t * NG : t * NG + 1]
                    )
                    c = 0
                    for e in range(1, n_dve):
                        nx = 1 - c
                        nc.vector.scalar_tensor_tensor(
                            out=a[nx],
                            in0=relu[:, slices[e]],
                            scalar=gn[:, t * NG + e : t * NG + e + 1],
                            in1=a[c],
                            op0=OP.mult,
                            op1=OP.add,
                        )
                        c = nx
                    h_dve = a[c]

                    # Pool: products for terms [n_dve, 8) via broadcast mult
                    ps = []
                    for e in range(n_dve, NG):
                        p = apool.tile(
                            [128, 128], acc_dt, name=f"pp{t}{e}", tag=f"pp{t}{e}"
                        )
                        r_in, g_in = bass.broadcast_tensor_aps(
                            relu[:, slices[e]],
                            gn[:, t * NG + e : t * NG + e + 1],
                        )
                        nc.gpsimd.tensor_tensor(out=p, in0=r_in, in1=g_in, op=OP.mult)
                        ps.append(p)
                    while len(ps) > 1:
                        q = apool.tile(
                            [128, 128],
                            acc_dt,
                            name=f"pq{t}{len(ps)}",
                            tag=f"pq{t}{len(ps)}",
                        )
                        nc.gpsimd.tensor_add(q, ps[0], ps[1])
                        ps = [q] + ps[2:]
                    # merge on Pool into the f32 tower
                    nc.gpsimd.tensor_add(tower[:, t, :], h_dve, ps[0])
                nc.sync.dma_start(
                    out=outs[i].rearrange("t b h -> b t h"), in_=tower
                )

    nc.compile()
    return nc


def _numpy_fallback(x, W_share, b_share, W_task, b_task, W_gate, b_gate):
    share = np.maximum(np.einsum("bd,edh->beh", x, W_share) + b_share, 0.0)
    task = np.maximum(
        np.einsum("bd,tedh->tbeh", x, W_task) + b_task[:, None], 0.0
    )
    logit = np.einsum("bd,tdg->tbg", x, W_gate) + b_gate[:, None]
    logit -= logit.max(axis=-1, keepdims=True)
    e = np.exp(logit)
    gate = e / e.sum(axis=-1, keepdims=True)
    share_b = np.broadcast_to(share[None], (N_TASK, x.shape[0], N_SHARE, H))
    experts = np.concatenate([share_b, task], axis=2)
    return np.einsum("tbeh,tbe->tbh", experts, gate).astype(np.float32)


def kernel(x, W_share, b_share, W_task, b_task, W_gate, b_gate):
    x = np.asarray(x, dtype=np.float32)
    W_share = np.asarray(W_share, dtype=np.float32)
    W_task = np.asarray(W_task, dtype=np.float32)
    W_gate = np.asarray(W_gate, dtype=np.float32)
    b_share = np.asarray(b_share, dtype=np.float32)
    b_task = np.asarray(b_task, dtype=np.float32)
    b_gate = np.asarray(b_gate, dtype=np.float32)

    if b_share.any() or b_task.any() or b_gate.any():
        # spec fills all biases with zeros; exact-but-slow fallback otherwise
        return _numpy_fallback(x, W_share, b_share, W_task, b_task, W_gate, b_gate)

    from concourse.bass_utils import run_bass_kernel_spmd

    if "nc" not in _CACHE:
        _CACHE["nc"] = _build_program()
    nc = _CACHE["nc"]

    # pack weights: [2 (k chunk), 128, 1552]
    wall = np.empty((2, 128, WCOLS), dtype=np.float32)
    for k in range(2):
        dk = slice(k * 128, (k + 1) * 128)
        wall[k, :, 0:512] = W_share.transpose(1, 0, 2).reshape(D_IN, 512)[dk]
        wall[k, :, 512:1024] = W_task[0].transpose(1, 0, 2).reshape(D_IN, 512)[dk]
        wall[k, :, 1024:1536] = W_task[1].transpose(1, 0, 2).reshape(D_IN, 512)[dk]
        wall[k, :, 1536 : 1536 + NG] = W_gate[0][dk]
        wall[k, :, 1536 + NG : WCOLS] = W_gate[1][dk]

    xT = np.ascontiguousarray(x.T)  # [256, 32768]

    in_maps = []
    for c in range(N_CORES):
        in_maps.append(
            {
                "xT": np.ascontiguousarray(xT[:, c * B_SHARD : (c + 1) * B_SHARD]),
                "wall": wall,
            }
        )

    res = run_bass_kernel_spmd(nc, in_maps, core_ids=list(range(N_CORES)))
    # per core: N_BLOCKS tensors out{i} of [2, 128, H] -> [2, 4096, H]
    per_core = [
        np.concatenate([r[f"out{i}"] for i in range(N_BLOCKS)], axis=1)
        for r in res.results
    ]
    return np.concatenate(per_core, axis=1)


# revision 57
# speedup vs baseline: 1.1062x; 1.0005x over previous
"""DMoE layer kernel for Trainium2 (8 NeuronCores, data-parallel over batch).

Computation (per task t in 0..1):
    share_e = relu(x @ W_share[e])            e in 0..3   (shared experts)
    task_te = relu(x @ W_task[t,e])           e in 0..3   (task experts)
    gate_t  = softmax(x @ W_gate[t], axis=-1)             (8 weights)
    towers[t] = sum_e gate[t,:,e] * concat([share, task_t])[:, e, :]

Layout strategy (per core, 4096 rows):
  - Host pre-transposes x -> xT [256, 4096] so no on-chip transpose is needed.
  - All weights packed host-side into W_all [2(k-chunk), 128, 1552]:
    cols 0:512 shared experts, 512:1024 task0, 1024:1536 task1, 1536:1552 gates.
  - Per 128-row block: stationary = xT chunk (float32r), moving = W_all
    (float32r) -> PSUM [128, 1552]; full fp32-class precision at 1 cyc/row.
  - ACT: one wide exp over both tasks' gate logits, one wide ReLU pass
    PSUM->SBUF (fp16), and the gate normalization (copy-with-scale).
  - DVE: per-task softmax denominators (tensor_reduce) + reciprocal; 13 of
    the 16 (task, expert) combine terms as fused mul-add chains
    (scalar_tensor_tensor: out = R_e * gn_te + acc, gate as per-partition
    scalar).
  - GpSimd: the remaining 3 combine terms as tensor_tensor mult with the
    gate column broadcast along the free dim, plus both merge adds into
    the f32 towers. (GpSimd cannot execute TensorScalarPtr on TRN2.)
"""

import numpy as np

B, D_IN, H = 32768, 256, 128
N_TASK, N_EXP, N_SHARE = 2, 4, 4
N_CORES = 8
B_SHARD = B // N_CORES          # 4096
N_BLOCKS = B_SHARD // 128       # 32
NG = N_SHARE + N_EXP            # 8 gate cols per task
WCOLS = 512 * 3 + 2 * NG        # 1552

_CACHE = {}


def _build_program(acc_dt_name: str = "float16"):
    import concourse.bass as bass
    import concourse.mybir as mybir
    import concourse.tile as tile
    from concourse import bacc

    f32 = mybir.dt.float32
    f32r = mybir.dt.float32r
    acc_dt = getattr(mybir.dt, acc_dt_name)
    AF = mybir.ActivationFunctionType
    OP = mybir.AluOpType

    nc = bacc.Bacc("TRN2", target_bir_lowering=False)
    xT = nc.dram_tensor("xT", [D_IN, B_SHARD], f32r, kind="ExternalInput")
    wall = nc.dram_tensor("wall", [2, 128, WCOLS], f32r, kind="ExternalInput")
    outs = [
        nc.dram_tensor(f"out{i}", [N_TASK, 128, H], f32, kind="ExternalOutput")
        for i in range(N_BLOCKS)
    ]

    # xT rows d -> (k chunk, p partition)
    xT_v = xT.rearrange("(k p) b -> p k b", k=2)
    wall_v = wall.rearrange("k p c -> p k c")

    with tile.TileContext(nc) as tc:
        with (
            tc.tile_pool(name="wsb", bufs=1) as wpool,
            tc.tile_pool(name="xsb", bufs=1) as xpool,
            tc.tile_pool(name="epsum", bufs=2, space="PSUM") as epool,
            tc.tile_pool(name="gpsum", bufs=2, space="PSUM") as gpool,
            tc.tile_pool(name="relu", bufs=8) as rpool,
            tc.tile_pool(name="small", bufs=16) as spool,
            tc.tile_pool(name="accs", bufs=10) as apool,
            tc.tile_pool(name="outs", bufs=10) as opool,
        ):
            w_sb = wpool.tile([128, 2, WCOLS], f32r)
            # split the weight load into per-k, per-column-group DMAs that
            # match the matmul consumers: the first matmul only waits on its
            # own 256KB chunk instead of the whole 1.6MB load
            # ACT exp-table warmup: the ~2.7us table load overlaps the
            # weight DMAs instead of landing on block 0's critical path
            warm = spool.tile([1, 1], f32, name="warm", tag="warm")
            nc.vector.memset(warm, 0.0)
            nc.scalar.activation(warm, warm, AF.Exp)

            # PE clock warmup: short matmuls on a const tile while the
            # weight DMAs stream, so block 0's real matmuls run warm
            pwarm = spool.tile([1, 128], f32, name="pwarm", tag="pwarm")
            nc.vector.memset(pwarm, 1.0)
            ps_w = epool.tile([1, 128], f32, name="ps_e", tag="ps_e")
            for _ in range(10):
                nc.tensor.matmul(
                    ps_w, pwarm[0:1, 0:1], pwarm, start=True, stop=True
                )

            # weight chunks split across the ACT HWDGE ring and the GpSimd
            # SWDGE (both idle at start) so they stream in parallel with the
            # x tiles on the SP ring; chunk order matches consumer order
            for idx, (k, (c0, c1)) in enumerate(
                (k, c)
                for k in range(2)
                for c in ((0, 512), (512, 1024), (1024, WCOLS))
            ):
                eng = nc.scalar if idx % 2 == 0 else nc.gpsimd
                eng.dma_start(out=w_sb[:, k, c0:c1], in_=wall_v[:, k, c0:c1])


            # front-load all x tiles (unique buffers, no deps): keeps the
            # SP DMA sequencer from head-of-line blocking later x loads
            # behind output DMAs that wait on compute.
            x_tiles = []
            for i in range(N_BLOCKS):
                x_sb = xpool.tile([128, 2, 128], f32r, name=f"x{i}", tag=f"x{i}")
                nc.sync.dma_start(out=x_sb, in_=xT_v[:, :, bass.ts(i, 128)])
                x_tiles.append(x_sb)

            for i in range(N_BLOCKS):
                bs = bass.ts(i, 128)
                x_sb = x_tiles[i]

                ps_e = epool.tile([128, 1536], f32)
                ps_g = gpool.tile([128, 2 * NG], f32)

                for k in range(2):
                    lhsT = x_sb[:, k, :]
                    nc.tensor.matmul(
                        ps_g,
                        lhsT,
                        w_sb[:, k, 1536:WCOLS],
                        start=(k == 0),
                        stop=(k == 1),
                    )
                    for j in range(3):
                        nc.tensor.matmul(
                            ps_e[:, bass.ts(j, 512)],
                            lhsT,
                            w_sb[:, k, bass.ts(j, 512)],
                            start=(k == 0),
                            stop=(k == 1),
                        )

                # gates: one wide exp on ACT; per-task denominators on DVE
                expS = spool.tile([128, 2 * NG], f32)
                nc.scalar.activation(expS, ps_g, AF.Exp)
                den = spool.tile([128, 2], f32)
                nc.vector.tensor_reduce(
                    den,
                    expS.rearrange("p (t g) -> p t g", t=2),
                    axis=mybir.AxisListType.X,
                    op=OP.add,
                )
                rden = spool.tile([128, 2], f32)
                nc.vector.reciprocal(rden, den)
                # normalized gates: gn[:, t*8:(t+1)*8] = expS_t * rden_t
                gn = spool.tile([128, 2 * NG], f32)
                for t in range(2):
                    nc.vector.tensor_scalar_mul(
                        gn[:, bass.ts(t, NG)],
                        expS[:, bass.ts(t, NG)],
                        rden[:, t : t + 1],
                    )

                # wide relu pass PSUM->SBUF, fp16, skipping the two tail
                # experts (they get fused relu+scale heads on ACT below)
                relu = rpool.tile([128, 1280], acc_dt)
                nc.scalar.activation(relu, ps_e[:, 0:1280], AF.Relu)

                # chain-head products on ACT: g*relu(x) == relu(g*x), g>0
                heads = [
                    apool.tile([128, 128], acc_dt, name=f"hd{t}", tag=f"hd{t}")
                    for t in range(2)
                ]
                for t in range(2):
                    nc.scalar.activation(
                        heads[t],
                        ps_e[:, 1280 + 128 * t : 1408 + 128 * t],
                        AF.Relu,
                        scale=gn[:, t * NG + 4 : t * NG + 5],
                    )

                # combine: towers[t] = sum_e gn_te * R_te
                # DVE: fused mul-add STT chains (1x, ~194ns/term) for 13 terms.
                # GpSimd (no TensorScalarPtr support on HW) takes 3 terms as
                # tensor_tensor mult(+add) with the gate column broadcast
                # along the free dim, plus both merge adds into the f32 tower.
                tower = opool.tile([128, 2, H], f32, name="tower", tag="tower")
                for t in range(2):
                    # relu-tile column of task-expert e (gate order):
                    # shared e0-3 at 128*e; task-specific e1-3 packed at
                    # 512+384*t; e4 (task-specific e0) lives on ACT heads
                    def col(e):
                        if e < 4:
                            return bass.ts(e, 128)
                        return bass.ds(512 + 384 * t + 128 * (e - 5), 128)

                    # DVE STT chain seeded by the ACT head product:
                    # t0: terms e0-3,e5,e6 (e7 on Pool)
                    # t1: terms e0-3,e5   (e6,e7 on Pool)
                    dve_terms = [0, 1, 2, 3, 5, 6] if t == 0 else [0, 1, 2, 3, 5]
                    a = [
                        apool.tile(
                            [128, 128], acc_dt, name=f"acc{t}{j}", tag=f"acc{t}{j}"
                        )
                        for j in range(2)
                    ]
                    prev = heads[t]
                    c = 0
                    for e in dve_terms:
                        nc.vector.scalar_tensor_tensor(
                            out=a[c],
                            in0=relu[:, col(e)],
                            scalar=gn[:, t * NG + e : t * NG + e + 1],
                            in1=prev,
                            op0=OP.mult,
                            op1=OP.add,
                        )
                        prev = a[c]
                        c = 1 - c
                    h_dve = prev

                    # Pool: remaining products via broadcast mult
                    pool_terms = [7] if t == 0 else [6, 7]
                    ps = []
                    for e in pool_terms:
                        p = apool.tile(
                            [128, 128], acc_dt, name=f"pp{t}{e}", tag=f"pp{t}{e}"
                        )
                        r_in, g_in = bass.broadcast_tensor_aps(
                            relu[:, col(e)],
                            gn[:, t * NG + e : t * NG + e + 1],
                        )
                        nc.gpsimd.tensor_tensor(out=p, in0=r_in, in1=g_in, op=OP.mult)
                        ps.append(p)
                    while len(ps) > 1:
                        q = apool.tile(
                            [128, 128],
                            acc_dt,
                            name=f"pq{t}{len(ps)}",
                            tag=f"pq{t}{len(ps)}",
                        )
                        nc.gpsimd.tensor_add(q, ps[0], ps[1])
                        ps = [q] + ps[2:]
                    # merge on Pool into the f32 tower
                    nc.gpsimd.tensor_add(tower[:, t, :], h_dve, ps[0])
                nc.sync.dma_start(
                    out=outs[i].rearrange("t b h -> b t h"), in_=tower
                )

    nc.compile()
    return nc


def _numpy_fallback(x, W_share, b_share, W_task, b_task, W_gate, b_gate):
    share = np.maximum(np.einsum("bd,edh->beh", x, W_share) + b_share, 0.0)
    task = np.maximum(
        np.einsum("bd,tedh->tbeh", x, W_task) + b_task[:, None], 0.0
    )
    logit = np.einsum("bd,tdg->tbg", x, W_gate) + b_gate[:, None]
    logit -= logit.max(axis=-1, keepdims=True)
    e = np.exp(logit)
    gate = e / e.sum(axis=-1, keepdims=True)
    share_b = np.broadcast_to(share[None], (N_TASK, x.shape[0], N_SHARE, H))
    experts = np.concatenate([share_b, task], axis=2)
    return np.einsum("tbeh,tbe->tbh", experts, gate).astype(np.float32)


def kernel(x, W_share, b_share, W_task, b_task, W_gate, b_gate):
    x = np.asarray(x, dtype=np.float32)
    W_share = np.asarray(W_share, dtype=np.float32)
    W_task = np.asarray(W_task, dtype=np.float32)
    W_gate = np.asarray(W_gate, dtype=np.float32)
    b_share = np.asarray(b_share, dtype=np.float32)
    b_task = np.asarray(b_task, dtype=np.float32)
    b_gate = np.asarray(b_gate, dtype=np.float32)

    if b_share.any() or b_task.any() or b_gate.any():
        # spec fills all biases with zeros; exact-but-slow fallback otherwise
        return _numpy_fallback(x, W_share, b_share, W_task, b_task, W_gate, b_gate)

    from concourse.bass_utils import run_bass_kernel_spmd

    if "nc" not in _CACHE:
        _CACHE["nc"] = _build_program()
    nc = _CACHE["nc"]

    # pack weights: [2 (k chunk), 128, 1552]
    # column layout: shared e0-3 | t0spec e1-3 | t1spec e1-3 | t0spec e0 |
    # t1spec e0 | gates.  The two *spec-e0 experts sit at the tail so the
    # device's wide ReLU can skip them (they get fused relu+scale on ACT).
    wall = np.empty((2, 128, WCOLS), dtype=np.float32)
    for k in range(2):
        dk = slice(k * 128, (k + 1) * 128)
        wall[k, :, 0:512] = W_share.transpose(1, 0, 2).reshape(D_IN, 512)[dk]
        wall[k, :, 512:896] = (
            W_task[0, 1:4].transpose(1, 0, 2).reshape(D_IN, 384)[dk]
        )
        wall[k, :, 896:1280] = (
            W_task[1, 1:4].transpose(1, 0, 2).reshape(D_IN, 384)[dk]
        )
        wall[k, :, 1280:1408] = W_task[0, 0][dk]
        wall[k, :, 1408:1536] = W_task[1, 0][dk]
        wall[k, :, 1536 : 1536 + NG] = W_gate[0][dk]
        wall[k, :, 1536 + NG : WCOLS] = W_gate[1][dk]

    xT = np.ascontiguousarray(x.T)  # [256, 32768]

    in_maps = []
    for c in range(N_CORES):
        in_maps.append(
            {
                "xT": np.ascontiguousarray(xT[:, c * B_SHARD : (c + 1) * B_SHARD]),
                "wall": wall,
            }
        )

    res = run_bass_kernel_spmd(nc, in_maps, core_ids=list(range(N_CORES)))
    # per core: N_BLOCKS tensors out{i} of [2, 128, H] -> [2, 4096, H]
    per_core = [
        np.concatenate([r[f"out{i}"] for i in range(N_BLOCKS)], axis=1)
        for r in res.results
    ]
    return np.concatenate(per_core, axis=1)
